# revision 8
# baseline (speedup 1.0000x reference)
"""Att-MIL pooling Trainium2 kernel (8-core SPMD, bags sharded across cores).

Reference computation (per bag of 64 instances):
    m   = max over D of x[N, L, 1, D]            -> [N, L]
    v   = tanh(m @ Wv.T + bv)                    -> [N, H]
    u   = sigmoid(m @ Wu.T + bu)                 -> [N, H]
    att = (v*u) @ Ww.T + bw                      -> [N]
    w   = softmax(att) within each bag           -> [N]
    out[b] = sum_i w_i * x_i                     -> [B, L, 1, D]

Sharding: 256 bags -> 32 bags/core (2048 instances/core), MLP weights
replicated. Each core processes its bags in 16 tiles of 128 instances
(= 2 bags), fully fused: one pass over x in SBUF serves both the
max-reduce and the weighted sum.
"""

import os
import sys

import numpy as np

# ---- problem constants (hardcoded; kernel.py must be self-contained) ----
B = 256          # bags
SZ = 64          # instances per bag
N = B * SZ       # 16384 instances
L = 64           # positions per instance
H = 128          # hidden
D = 256          # feature dim reduced by max
F = L * D        # 16384 floats per instance

NCORES = 8
BPC = B // NCORES          # 32 bags per core
IPC = BPC * SZ             # 2048 instances per core
TI = 128                   # instances per tile = 2 bags
BAGS_PER_TILE = TI // SZ   # 2
NT = IPC // TI             # 16 tiles per core
FH = F // 2                # 8192: half-tile free size (SBUF budget)

# matmul dtype for the big weighted-sum matmuls:
#   "f32r" -> float32r (1 cycle/row on PE, slightly relaxed precision)
#   "f32"  -> float32  (4 cycles/row, exact)
MM_MODE = os.environ.get("MIL_MM_MODE", "f32r")

_CACHE = {}


def _build_nc():
    import concourse.bass as bass
    import concourse.bacc as bacc
    import concourse.tile as tile
    from concourse import mybir

    f32 = mybir.dt.float32
    nc = bacc.Bacc(None, target_bir_lowering=False)

    x = nc.declare_dram_parameter("x", [IPC, F], f32, isOutput=False)
    wvt = nc.declare_dram_parameter("wvt", [L, H], f32, isOutput=False)
    wut = nc.declare_dram_parameter("wut", [L, H], f32, isOutput=False)
    wwt = nc.declare_dram_parameter("wwt", [H, 1], f32, isOutput=False)
    bv = nc.declare_dram_parameter("bv", [H, 1], f32, isOutput=False)
    buh = nc.declare_dram_parameter("buh", [H, 1], f32, isOutput=False)
    ident = nc.declare_dram_parameter("ident", [128, 128], f32, isOutput=False)
    wmask = nc.declare_dram_parameter(
        "wmask", [BAGS_PER_TILE, TI], f32, isOutput=False
    )
    out = nc.declare_dram_parameter("out", [BPC, F], f32, isOutput=True)
    attn = nc.declare_dram_parameter("attn", [IPC], f32, isOutput=True)

    mm_dt = {"f32r": mybir.dt.float32r, "f32": f32}[MM_MODE]

    def mm_ap(ap):
        return ap.bitcast(mm_dt) if mm_dt is not f32 else ap

    with tile.TileContext(nc) as tc:
        with (
            tc.tile_pool(name="consts", bufs=1) as consts,
            tc.tile_pool(name="xpool", bufs=3) as xpool,
            tc.tile_pool(name="small", bufs=2) as small,
            tc.tile_pool(name="obuf", bufs=2) as opool,
            tc.tile_pool(name="ps_small", bufs=1, space="PSUM") as ps_small,
            tc.tile_pool(name="ps_acc", bufs=3, space="PSUM") as ps_acc,
        ):
            ident_sb = consts.tile([128, 128], f32)
            nc.sync.dma_start(out=ident_sb, in_=ident[:, :])
            wvt_sb = consts.tile([L, H], f32)
            nc.sync.dma_start(out=wvt_sb, in_=wvt[:, :])
            wut_sb = consts.tile([L, H], f32)
            nc.sync.dma_start(out=wut_sb, in_=wut[:, :])
            wwt_sb = consts.tile([H, 1], f32)
            nc.sync.dma_start(out=wwt_sb, in_=wwt[:, :])
            bv_sb = consts.tile([H, 1], f32)
            nc.sync.dma_start(out=bv_sb, in_=bv[:, :])
            buh_sb = consts.tile([H, 1], f32)
            nc.sync.dma_start(out=buh_sb, in_=buh[:, :])
            wmask_sb = consts.tile([BAGS_PER_TILE, TI], f32)
            nc.sync.dma_start(out=wmask_sb, in_=wmask[:, :])

            for t in range(NT):
                r0 = t * TI
                # --- load the 128-instance tile in two halves ---
                xh = []
                for h in range(2):
                    xt = xpool.tile([TI, FH], f32, tag="xt")
                    nc.sync.dma_start(
                        out=xt, in_=x[r0 : r0 + TI, h * FH : (h + 1) * FH]
                    )
                    xh.append(xt)

                # --- m = max over D ---  [128, 64]
                m_sb = small.tile([TI, L], f32, tag="m")
                for h in range(2):
                    nc.vector.reduce_max(
                        out=m_sb[:, h * (L // 2) : (h + 1) * (L // 2)],
                        in_=xh[h].rearrange("p (l d) -> p l d", d=D),
                        axis=mybir.AxisListType.X,
                    )

                # --- mT = m.T ---  [64, 128] via PE transpose
                mT_ps = ps_small.tile([L, TI], f32, tag="mT")
                nc.tensor.transpose(mT_ps, m_sb, ident_sb)
                mT_sb = small.tile([L, TI], f32, tag="mT_sb")
                nc.scalar.copy(out=mT_sb, in_=mT_ps)

                # --- MLP: v = tanh(Wv @ mT + bv); u = sigmoid(Wu @ mT + bu)
                # sigmoid(z) = 0.5*tanh(z/2) + 0.5 (keeps ACT on one table set)
                pv_ps = ps_small.tile([H, TI], f32, tag="pv")
                nc.tensor.matmul(pv_ps, wvt_sb, mT_sb, start=True, stop=True)
                pu_ps = ps_small.tile([H, TI], f32, tag="pu")
                nc.tensor.matmul(pu_ps, wut_sb, mT_sb, start=True, stop=True)
                v_sb = small.tile([H, TI], f32, tag="v")
                nc.scalar.activation(
                    out=v_sb, in_=pv_ps,
                    func=mybir.ActivationFunctionType.Tanh,
                    bias=bv_sb, scale=1.0,
                )
                th_sb = small.tile([H, TI], f32, tag="th")
                nc.scalar.activation(
                    out=th_sb, in_=pu_ps,
                    func=mybir.ActivationFunctionType.Tanh,
                    bias=buh_sb, scale=0.5,
                )
                # g = v * (0.5*th + 0.5) = (th + 1) * 0.5 * v
                t1_sb = small.tile([H, TI], f32, tag="t1")
                nc.vector.tensor_scalar_add(out=t1_sb, in0=th_sb, scalar1=1.0)
                g_sb = small.tile([H, TI], f32, tag="g")
                nc.vector.scalar_tensor_tensor(
                    out=g_sb, in0=t1_sb, scalar=0.5, in1=v_sb,
                    op0=mybir.AluOpType.mult, op1=mybir.AluOpType.mult,
                )

                # --- att = Ww @ g ---  [1, 128]  (bw dropped: softmax-invariant)
                att_ps = ps_small.tile([1, TI], f32, tag="att")
                nc.tensor.matmul(att_ps, wwt_sb, g_sb, start=True, stop=True)
                att_sb = small.tile([1, TI], f32, tag="att_sb")
                nc.scalar.copy(out=att_sb, in_=att_ps)

                # --- reshape att [1, 128] -> [2, 64] (bag per partition) ---
                attb = small.tile([BAGS_PER_TILE, SZ], f32, tag="attb")
                nc.sync.dma_start(out=attb, in_=att_sb)

                # --- per-bag softmax ---
                negmax = small.tile([BAGS_PER_TILE, 1], f32, tag="negmax")
                nc.vector.reduce_max(
                    out=negmax, in_=attb, axis=mybir.AxisListType.X, negate=True
                )
                e_sb = small.tile([BAGS_PER_TILE, SZ], f32, tag="e")
                den = small.tile([BAGS_PER_TILE, 1], f32, tag="den")
                nc.scalar.activation(
                    out=e_sb, in_=attb,
                    func=mybir.ActivationFunctionType.Exp,
                    bias=negmax, scale=1.0, accum_out=den,
                )
                rec = small.tile([BAGS_PER_TILE, 1], f32, tag="rec")
                nc.vector.reciprocal(out=rec, in_=den)
                w_sb = small.tile([BAGS_PER_TILE, SZ], f32, tag="w")
                nc.vector.tensor_scalar_mul(out=w_sb, in0=e_sb, scalar1=rec)
                nc.sync.dma_start(out=attn[r0 : r0 + TI], in_=w_sb)

                # --- build block-diagonal weight matrix [128, 2] ---
                # wpad[0, 0:64] = w[0]; wpad[1, 64:128] = w[1]; rest zero.
                # wpad[p, b, c] = w[p, c] * wmask[p, b, c] (w broadcast over b)
                wpad = small.tile([BAGS_PER_TILE, TI], f32, tag="wpad")
                wap = w_sb[:, :]
                w_bc = bass.AP(
                    tensor=wap.tensor,
                    offset=wap.offset,
                    ap=[wap.ap[0], [0, BAGS_PER_TILE], wap.ap[1]],
                )
                nc.vector.tensor_mul(
                    out=wpad.rearrange("p (b c) -> p b c", b=BAGS_PER_TILE),
                    in0=w_bc,
                    in1=wmask_sb.rearrange("p (b c) -> p b c", b=BAGS_PER_TILE),
                )
                wblk_ps = ps_small.tile([TI, BAGS_PER_TILE], f32, tag="wblk")
                nc.tensor.transpose(
                    wblk_ps, wpad, ident_sb[:BAGS_PER_TILE, :BAGS_PER_TILE]
                )
                wblk_sb = small.tile([TI, BAGS_PER_TILE], f32, tag="wblk_sb")
                nc.scalar.copy(out=wblk_sb, in_=wblk_ps)

                # --- weighted sum: out[2, 16384] = wblk.T @ x ---
                for h in range(2):
                    ob = opool.tile([BAGS_PER_TILE, FH], f32, tag="ob")
                    for j in range(FH // 512):
                        acc = ps_acc.tile([BAGS_PER_TILE, 512], f32, tag="acc")
                        nc.tensor.matmul(
                            acc,
                            mm_ap(wblk_sb),
                            mm_ap(xh[h][:, j * 512 : (j + 1) * 512]),
                            start=True, stop=True,
                        )
                        nc.scalar.copy(
                            out=ob[:, j * 512 : (j + 1) * 512], in_=acc
                        )
                    nc.sync.dma_start(
                        out=out[
                            t * BAGS_PER_TILE : (t + 1) * BAGS_PER_TILE,
                            h * FH : (h + 1) * FH,
                        ],
                        in_=ob,
                    )
    nc.finalize()
    return nc


def _prep_weight_maps(Wv, bv, Wu, bu, Ww):
    wvt = np.ascontiguousarray(Wv.T, dtype=np.float32)          # [L, H]
    wut = np.ascontiguousarray(Wu.T, dtype=np.float32)          # [L, H]
    wwt = np.ascontiguousarray(Ww.reshape(1, H).T, np.float32)  # [H, 1]
    bvc = np.ascontiguousarray(bv.reshape(H, 1), np.float32)
    buh = np.ascontiguousarray((0.5 * bu).reshape(H, 1), np.float32)
    ident = np.eye(128, dtype=np.float32)
    wmask = np.zeros((BAGS_PER_TILE, TI), np.float32)
    for b in range(BAGS_PER_TILE):
        wmask[b, b * SZ : (b + 1) * SZ] = 1.0
    return wvt, wut, wwt, bvc, buh, ident, wmask


def _make_in_maps(inter_pre, Wv, bv, Wu, bu, Ww):
    wvt, wut, wwt, bvc, buh, ident, wmask = _prep_weight_maps(Wv, bv, Wu, bu, Ww)
    xf = np.ascontiguousarray(inter_pre, dtype=np.float32).reshape(N, F)
    in_maps = []
    for c in range(NCORES):
        in_maps.append({
            "x": xf[c * IPC : (c + 1) * IPC],
            "wvt": wvt, "wut": wut, "wwt": wwt,
            "bv": bvc, "buh": buh, "ident": ident, "wmask": wmask,
        })
    return in_maps


def _numpy_fallback(inter_pre, Wv, bv, Wu, bu, Ww, bw, bags_size):
    """Exact numpy implementation for ragged bag sizes (safety net)."""
    n = inter_pre.shape[0]
    b = bags_size.shape[0]
    seg = np.repeat(np.arange(b), bags_size)[:n]
    m = inter_pre.max(axis=-1).reshape(n, -1)
    v = np.tanh(m @ Wv.T + bv)
    u = 1.0 / (1.0 + np.exp(-(m @ Wu.T + bu)))
    att = ((v * u) @ Ww.T + bw).reshape(n)
    seg_max = np.full(b, -np.inf, np.float32)
    np.maximum.at(seg_max, seg, att)
    e = np.exp(att - seg_max[seg])
    den = np.zeros(b, np.float32)
    np.add.at(den, seg, e)
    w = (e / den[seg]).astype(np.float32)
    x = inter_pre.reshape(n, -1)
    bag = np.zeros((b, x.shape[1]), np.float32)
    np.add.at(bag, seg, w[:, None] * x)
    Lc, Dc = inter_pre.shape[1], inter_pre.shape[3]
    return bag.reshape(b, Lc, 1, Dc).astype(np.float32), w


def kernel(inter_pre, Wv, bv, Wu, bu, Ww, bw, bags_size):
    inter_pre = np.asarray(inter_pre)
    bags_size = np.asarray(bags_size)
    if not (
        bags_size.shape == (B,)
        and np.all(bags_size == SZ)
        and inter_pre.shape == (N, L, 1, D)
    ):
        return _numpy_fallback(
            np.asarray(inter_pre, np.float32),
            np.asarray(Wv, np.float32), np.asarray(bv, np.float32),
            np.asarray(Wu, np.float32), np.asarray(bu, np.float32),
            np.asarray(Ww, np.float32), np.asarray(bw, np.float32),
            bags_size,
        )

    from concourse.bass_utils import run_bass_kernel_spmd

    if "nc" not in _CACHE:
        _CACHE["nc"] = _build_nc()
    nc = _CACHE["nc"]

    in_maps = _make_in_maps(inter_pre, Wv, bv, Wu, bu, Ww)
    res = run_bass_kernel_spmd(nc, in_maps, list(range(NCORES)))

    debag = np.concatenate(
        [res.results[c]["out"] for c in range(NCORES)], axis=0
    ).reshape(B, L, 1, D)
    w = np.concatenate([res.results[c]["attn"] for c in range(NCORES)])
    return debag.astype(np.float32), w.astype(np.float32)


# revision 16
# speedup vs baseline: 52.3697x; 52.3697x over previous
"""Att-MIL pooling Trainium2 kernel (8-core SPMD, bags sharded across cores).

Reference computation (per bag of 64 instances):
    m   = max over D of x[N, L, 1, D]            -> [N, L]
    v   = tanh(m @ Wv.T + bv)                    -> [N, H]
    u   = sigmoid(m @ Wu.T + bu)                 -> [N, H]
    att = (v*u) @ Ww.T + bw                      -> [N]
    w   = softmax(att) within each bag           -> [N]
    out[b] = sum_i w_i * x_i                     -> [B, L, 1, D]

Sharding: 256 bags -> 32 bags/core (2048 instances/core), MLP weights
replicated. Each core processes its bags in 16 tiles of 128 instances
(= 2 bags), fully fused: one pass over x in SBUF serves both the
max-reduce and the weighted sum.
"""

import os
import sys

import numpy as np

# ---- problem constants (hardcoded; kernel.py must be self-contained) ----
B = 256          # bags
SZ = 64          # instances per bag
N = B * SZ       # 16384 instances
L = 64           # positions per instance
H = 128          # hidden
D = 256          # feature dim reduced by max
F = L * D        # 16384 floats per instance

NCORES = 8
BPC = B // NCORES          # 32 bags per core
IPC = BPC * SZ             # 2048 instances per core
TI = 128                   # instances per tile = 2 bags
BAGS_PER_TILE = TI // SZ   # 2
NT = IPC // TI             # 16 tiles per core
FH = F // 2                # 8192: half-tile free size (SBUF budget)

# matmul dtype for the big weighted-sum matmuls:
#   "f32r" -> float32r (1 cycle/row on PE, slightly relaxed precision)
#   "f32"  -> float32  (4 cycles/row, exact)
MM_MODE = os.environ.get("MIL_MM_MODE", "f32r")
# perf probing: repeat the whole per-core computation PASSES times inside the
# NEFF (measures steady-state device time without profiling infrastructure)
PASSES = int(os.environ.get("MIL_PASSES", "1"))

_CACHE = {}


def _build_nc():
    import concourse.bass as bass
    import concourse.bacc as bacc
    import concourse.tile as tile
    from concourse import mybir

    f32 = mybir.dt.float32
    # In f32r mode, x lives in DRAM/SBUF as float32r (same bits as f32).
    # HWDGE loads need no cast; the weighted-sum matmul then runs at
    # 1 cycle/row instead of fp32's 4. The max-reduce reads the same tile
    # (values rounded to f32r precision ~1e-4, only perturbing softmax
    # logits). wblk is rounded to f32r by a DVE copy.
    xdt = mybir.dt.float32r if MM_MODE == "f32r" else f32
    nc = bacc.Bacc(None, target_bir_lowering=False)

    x = nc.declare_dram_parameter("x", [IPC, F], xdt, isOutput=False)
    wvt = nc.declare_dram_parameter("wvt", [L, H], f32, isOutput=False)
    wut = nc.declare_dram_parameter("wut", [L, H], f32, isOutput=False)
    wwt = nc.declare_dram_parameter("wwt", [H, 1], f32, isOutput=False)
    bv = nc.declare_dram_parameter("bv", [H, 1], f32, isOutput=False)
    buh = nc.declare_dram_parameter("buh", [H, 1], f32, isOutput=False)
    ident = nc.declare_dram_parameter("ident", [128, 128], f32, isOutput=False)
    wmask = nc.declare_dram_parameter(
        "wmask", [BAGS_PER_TILE, TI], f32, isOutput=False
    )
    out = nc.declare_dram_parameter("out", [BPC, F], f32, isOutput=True)
    attn = nc.declare_dram_parameter("attn", [IPC], f32, isOutput=True)


    with tile.TileContext(nc) as tc:
        with (
            tc.tile_pool(name="consts", bufs=1) as consts,
            tc.tile_pool(name="xpool", bufs=4) as xpool,
            tc.tile_pool(name="small", bufs=2) as small,
            tc.tile_pool(name="obuf", bufs=2) as opool,
            tc.tile_pool(name="ps_small", bufs=1, space="PSUM") as ps_small,
            tc.tile_pool(name="ps_acc", bufs=2, space="PSUM") as ps_acc,
            tc.tile_pool(name="ps_tr", bufs=1, space="PSUM") as ps_tr,
        ):
            ident_sb = consts.tile([128, 128], f32)
            nc.sync.dma_start(out=ident_sb, in_=ident[:, :])
            wvt_sb = consts.tile([L, H], f32)
            nc.sync.dma_start(out=wvt_sb, in_=wvt[:, :])
            wut_sb = consts.tile([L, H], f32)
            nc.sync.dma_start(out=wut_sb, in_=wut[:, :])
            wwt_sb = consts.tile([H, 1], f32)
            nc.sync.dma_start(out=wwt_sb, in_=wwt[:, :])
            bv_sb = consts.tile([H, 1], f32)
            nc.sync.dma_start(out=bv_sb, in_=bv[:, :])
            buh_sb = consts.tile([H, 1], f32)
            nc.sync.dma_start(out=buh_sb, in_=buh[:, :])
            wmask_sb = consts.tile([BAGS_PER_TILE, TI], f32)
            nc.sync.dma_start(out=wmask_sb, in_=wmask[:, :])
            attn_sb = consts.tile([BAGS_PER_TILE, NT * SZ], f32)

            for it in range(PASSES * NT):
                t = it % NT
                r0 = t * TI
                # --- load the 128-instance tile in two halves, alternating
                # between the two HWDGE rings (SP and ACT) so big loads from
                # consecutive tiles overlap ---
                xh = []
                for h in range(2):
                    xt = xpool.tile([TI, FH], xdt, tag="xt")
                    ring = nc.sync if (2 * t + h) % 2 == 0 else nc.gpsimd
                    ring.dma_start(
                        out=xt, in_=x[r0 : r0 + TI, h * FH : (h + 1) * FH]
                    )
                    xh.append(xt)

                # --- m = max over D ---  [128, 64]
                m_sb = small.tile([TI, L], f32, tag="m")
                for h in range(2):
                    nc.vector.reduce_max(
                        out=m_sb[:, h * (L // 2) : (h + 1) * (L // 2)],
                        in_=xh[h].rearrange("p (l d) -> p l d", d=D),
                        axis=mybir.AxisListType.X,
                    )

                # --- mT = m.T ---  [64, 128] via PE transpose
                mT_ps = ps_small.tile([L, TI], f32, tag="mT")
                nc.tensor.transpose(mT_ps, m_sb, ident_sb)
                mT_sb = small.tile([L, TI], f32, tag="mT_sb")
                nc.scalar.copy(out=mT_sb, in_=mT_ps)

                # --- MLP: v = tanh(Wv @ mT + bv); u = sigmoid(Wu @ mT + bu)
                # sigmoid(z) = 0.5*tanh(z/2) + 0.5 (keeps ACT on one table set)
                pv_ps = ps_small.tile([H, TI], f32, tag="pv")
                nc.tensor.matmul(pv_ps, wvt_sb, mT_sb, start=True, stop=True)
                pu_ps = ps_small.tile([H, TI], f32, tag="pu")
                nc.tensor.matmul(pu_ps, wut_sb, mT_sb, start=True, stop=True)
                v_sb = small.tile([H, TI], f32, tag="v")
                nc.scalar.activation(
                    out=v_sb, in_=pv_ps,
                    func=mybir.ActivationFunctionType.Tanh,
                    bias=bv_sb, scale=1.0,
                )
                th_sb = small.tile([H, TI], f32, tag="th")
                nc.scalar.activation(
                    out=th_sb, in_=pu_ps,
                    func=mybir.ActivationFunctionType.Tanh,
                    bias=buh_sb, scale=0.5,
                )
                # g = v * (0.5*th + 0.5) = (th + 1) * 0.5 * v
                t1_sb = small.tile([H, TI], f32, tag="t1")
                nc.scalar.add(out=t1_sb, in_=th_sb, add=1.0)
                g_sb = small.tile([H, TI], f32, tag="g")
                nc.vector.scalar_tensor_tensor(
                    out=g_sb, in0=t1_sb, scalar=0.5, in1=v_sb,
                    op0=mybir.AluOpType.mult, op1=mybir.AluOpType.mult,
                )

                # --- att = Ww @ g ---  [1, 128]  (bw dropped: softmax-invariant)
                att_ps = ps_small.tile([1, TI], f32, tag="att")
                nc.tensor.matmul(att_ps, wwt_sb, g_sb, start=True, stop=True)
                att_sb = small.tile([1, TI], f32, tag="att_sb")
                nc.scalar.copy(out=att_sb, in_=att_ps)

                # --- reshape att [1, 128] -> [2, 64] (bag per partition) ---
                attb = small.tile([BAGS_PER_TILE, SZ], f32, tag="attb")
                nc.gpsimd.dma_start(out=attb, in_=att_sb)

                # --- per-bag softmax ---
                negmax = small.tile([BAGS_PER_TILE, 1], f32, tag="negmax")
                nc.vector.reduce_max(
                    out=negmax, in_=attb, axis=mybir.AxisListType.X, negate=True
                )
                e_sb = small.tile([BAGS_PER_TILE, SZ], f32, tag="e")
                den = small.tile([BAGS_PER_TILE, 1], f32, tag="den")
                nc.scalar.activation(
                    out=e_sb, in_=attb,
                    func=mybir.ActivationFunctionType.Exp,
                    bias=negmax, scale=1.0, accum_out=den,
                )
                rec = small.tile([BAGS_PER_TILE, 1], f32, tag="rec")
                nc.vector.reciprocal(out=rec, in_=den)
                w_sb = small.tile([BAGS_PER_TILE, SZ], f32, tag="w")
                nc.vector.tensor_scalar_mul(out=w_sb, in0=e_sb, scalar1=rec)
                # stash softmax weights; one merged DMA to attn at the end
                nc.vector.tensor_copy(
                    out=attn_sb[:, t * SZ : (t + 1) * SZ], in_=w_sb
                )

                # --- build block-diagonal weight matrix [128, 2] ---
                # wpad[0, 0:64] = w[0]; wpad[1, 64:128] = w[1]; rest zero.
                # wpad[p, b, c] = w[p, c] * wmask[p, b, c] (w broadcast over b)
                wpad = small.tile([BAGS_PER_TILE, TI], f32, tag="wpad")
                wap = w_sb[:, :]
                w_bc = bass.AP(
                    tensor=wap.tensor,
                    offset=wap.offset,
                    ap=[wap.ap[0], [0, BAGS_PER_TILE], wap.ap[1]],
                )
                nc.vector.tensor_mul(
                    out=wpad.rearrange("p (b c) -> p b c", b=BAGS_PER_TILE),
                    in0=w_bc,
                    in1=wmask_sb.rearrange("p (b c) -> p b c", b=BAGS_PER_TILE),
                )
                wblk_ps = ps_small.tile([TI, BAGS_PER_TILE], f32, tag="wblk")
                nc.tensor.transpose(
                    wblk_ps, wpad, ident_sb[:BAGS_PER_TILE, :BAGS_PER_TILE]
                )
                wblk_sb = small.tile([TI, BAGS_PER_TILE], xdt, tag="wblk_sb")
                nc.vector.tensor_copy(out=wblk_sb, in_=wblk_ps)

                # --- weighted sum, X-stationary orientation:
                # out[feat, bag] = sum_i x[i, feat] * wblk[i, bag].
                # 64 chunk-matmuls per half fill acc[:, b*64 + j]; the result
                # lands feature-major on 128 partitions (cheap evacuation),
                # then per-bag PE transposes restore DRAM layout.
                tr_sb = opool.tile([SZ, 2 * BAGS_PER_TILE, TI], f32, tag="tr")
                for h in range(2):
                    acc = ps_acc.tile([TI, TI], f32, tag="acc")
                    for j in range(FH // TI):
                        nc.tensor.matmul(
                            acc[:, 2 * j : 2 * j + 2],
                            xh[h][:, j * TI : (j + 1) * TI],
                            wblk_sb,
                            start=True, stop=True,
                        )
                    # deinterleave (j, b) -> (b, j) while evacuating PSUM
                    acc_sb = small.tile([TI, TI], f32, tag="acc_sb")
                    nc.scalar.copy(
                        out=acc_sb.rearrange("p (b j) -> p b j", b=BAGS_PER_TILE),
                        in_=acc.rearrange("p (j b) -> p b j", b=BAGS_PER_TILE),
                    )
                    for b in range(BAGS_PER_TILE):
                        tr_ps = ps_tr.tile([SZ, TI], f32, tag="tr_ps")
                        nc.tensor.transpose(
                            tr_ps, acc_sb[:, b * SZ : (b + 1) * SZ], ident_sb
                        )
                        nc.scalar.copy(out=tr_sb[:, b * 2 + h, :], in_=tr_ps)
                # one DMA per tile: out[2t+b, h*FH + j*TI + p] <- tr_sb[j, b, h, p]
                dst = bass.AP(
                    tensor=out,
                    offset=t * BAGS_PER_TILE * F,
                    ap=[[TI, SZ], [F, BAGS_PER_TILE], [FH, 2], [1, TI]],
                )
                nc.sync.dma_start(out=dst, in_=tr_sb.rearrange(
                    "j bh p -> j bh p"))

            # merged attn store: attn[t*128 + b*64 + i] <- attn_sb[b, t*64+i]
            attn_dst = bass.AP(
                tensor=attn,
                offset=0,
                ap=[[SZ, BAGS_PER_TILE], [TI, NT], [1, SZ]],
            )
            nc.sync.dma_start(
                out=attn_dst,
                in_=attn_sb.rearrange("b (t i) -> b t i", t=NT),
            )
    nc.finalize()
    return nc


def _prep_weight_maps(Wv, bv, Wu, bu, Ww):
    wvt = np.ascontiguousarray(Wv.T, dtype=np.float32)          # [L, H]
    wut = np.ascontiguousarray(Wu.T, dtype=np.float32)          # [L, H]
    wwt = np.ascontiguousarray(Ww.reshape(1, H).T, np.float32)  # [H, 1]
    bvc = np.ascontiguousarray(bv.reshape(H, 1), np.float32)
    buh = np.ascontiguousarray((0.5 * bu).reshape(H, 1), np.float32)
    ident = np.eye(128, dtype=np.float32)
    wmask = np.zeros((BAGS_PER_TILE, TI), np.float32)
    for b in range(BAGS_PER_TILE):
        wmask[b, b * SZ : (b + 1) * SZ] = 1.0
    return wvt, wut, wwt, bvc, buh, ident, wmask


def _make_in_maps(inter_pre, Wv, bv, Wu, bu, Ww):
    wvt, wut, wwt, bvc, buh, ident, wmask = _prep_weight_maps(Wv, bv, Wu, bu, Ww)
    xf = np.ascontiguousarray(inter_pre, dtype=np.float32).reshape(N, F)
    in_maps = []
    for c in range(NCORES):
        in_maps.append({
            "x": xf[c * IPC : (c + 1) * IPC],
            "wvt": wvt, "wut": wut, "wwt": wwt,
            "bv": bvc, "buh": buh, "ident": ident, "wmask": wmask,
        })
    return in_maps


def _numpy_fallback(inter_pre, Wv, bv, Wu, bu, Ww, bw, bags_size):
    """Exact numpy implementation for ragged bag sizes (safety net)."""
    n = inter_pre.shape[0]
    b = bags_size.shape[0]
    seg = np.repeat(np.arange(b), bags_size)[:n]
    m = inter_pre.max(axis=-1).reshape(n, -1)
    v = np.tanh(m @ Wv.T + bv)
    u = 1.0 / (1.0 + np.exp(-(m @ Wu.T + bu)))
    att = ((v * u) @ Ww.T + bw).reshape(n)
    seg_max = np.full(b, -np.inf, np.float32)
    np.maximum.at(seg_max, seg, att)
    e = np.exp(att - seg_max[seg])
    den = np.zeros(b, np.float32)
    np.add.at(den, seg, e)
    w = (e / den[seg]).astype(np.float32)
    x = inter_pre.reshape(n, -1)
    bag = np.zeros((b, x.shape[1]), np.float32)
    np.add.at(bag, seg, w[:, None] * x)
    Lc, Dc = inter_pre.shape[1], inter_pre.shape[3]
    return bag.reshape(b, Lc, 1, Dc).astype(np.float32), w


def kernel(inter_pre, Wv, bv, Wu, bu, Ww, bw, bags_size):
    inter_pre = np.asarray(inter_pre)
    bags_size = np.asarray(bags_size)
    if not (
        bags_size.shape == (B,)
        and np.all(bags_size == SZ)
        and inter_pre.shape == (N, L, 1, D)
    ):
        return _numpy_fallback(
            np.asarray(inter_pre, np.float32),
            np.asarray(Wv, np.float32), np.asarray(bv, np.float32),
            np.asarray(Wu, np.float32), np.asarray(bu, np.float32),
            np.asarray(Ww, np.float32), np.asarray(bw, np.float32),
            bags_size,
        )

    from concourse.bass_utils import run_bass_kernel_spmd

    if "nc" not in _CACHE:
        _CACHE["nc"] = _build_nc()
    nc = _CACHE["nc"]

    in_maps = _make_in_maps(inter_pre, Wv, bv, Wu, bu, Ww)
    res = run_bass_kernel_spmd(nc, in_maps, list(range(NCORES)))

    debag = np.concatenate(
        [res.results[c]["out"] for c in range(NCORES)], axis=0
    ).reshape(B, L, 1, D)
    w = np.concatenate([res.results[c]["attn"] for c in range(NCORES)])
    return debag.astype(np.float32), w.astype(np.float32)
